# revision 34
# baseline (speedup 1.0000x reference)
"""Multi-head attention (B=2, S=2048, E=1024, H=16) on 8 Trainium2 NeuronCores.

Sharding: data-parallel over the 2 batches x tensor-parallel over 4 head-groups
(4 heads each).  Core c handles batch c//4, heads [4*(c%4), 4*(c%4)+4).
Each core computes its heads' Q/K/V projections, softmax(QK^T/8)V, and the
partial output projection against the matching Wo column slice; the host sums
the 4 partials per batch (the head-parallel all-reduce) and stacks batches.

Device-side design (bf16 everywhere, fp32 PSUM accumulate):
 - The kernel is scalar-engine bound: 128 exp tiles of [128,1024] at ~1.15us
   each is ~147us that nothing else can absorb (DVE has no exp).  Everything
   is scheduled so the ACT chain never stalls:
   * scores for iteration n+1 issue on the PE BEFORE the PV matmuls of
     iteration n (PV waits on ACT(n), scores don't), so the in-order PE
     stream can't starve the scalar engine.
   * projection/epilogue work is doled out from a deadline-stamped queue at
     a bounded rate (one ~1us chain-half only when its deadline nears or
     every few iterations) so iterations stay under the ACT period.
 - Host pre-packs every dram tensor in its exact SBUF tile layout, so each
   input DMA is one contiguous descriptor per partition (~0.7us issue,
   full bandwidth), and the first-needed tensors are issued first across
   three different engine queues.
 - Scores are produced transposed, sT[j, i] = k_j . q_i; head pairs sit in
   PE row groups 0-63/64-127 so their K=64 score matmuls run concurrently.
   softmax's partition-dim sum rides the PV matmul as a ones column on V.
 - The attention PSUM bank is drained by ONE [65,1024] copy per head pair;
   the softmax reciprocal (slow on DVE) is stamped a few iterations later so
   its dependent broadcast-matmul never blocks the PE stream.
"""

import numpy as np
import ml_dtypes

import concourse.bass as bass
from concourse import bacc
import concourse.mybir as mybir
import concourse.tile as tile
from concourse.bass_utils import run_bass_kernel_spmd

B, S, E, H = 2, 2048, 1024, 16
DK = 64
NCORES = 8
HGROUPS = 4            # head-parallel groups per batch
HLOC = H // HGROUPS    # heads per core = 4
FH = HLOC * DK         # local feature cols = 256

EC = E // 128        # 8 contraction chunks for the projections
ST = S // 128        # 16 seq tiles of 128 (the j tiles)
SC = S // 512        # 4 seq chunks of 512 (the i chunks)
FT = FH // 128       # 2 feature tiles (head pairs)

F32 = mybir.dt.float32
BF16 = mybir.dt.bfloat16
EXP_BIAS = -4.0        # constant shift inside exp; cancels in softmax


def _build_program() -> bass.Bass:
    nc = bacc.Bacc("TRN2", target_bir_lowering=False, debug=False,
                   enable_asserts=False)

    # all inputs are pre-packed on the host into the exact SBUF tile layout,
    # so every DMA below is contiguous.
    xt_d = nc.dram_tensor("xt", [SC, 128, EC * 512], BF16,
                          kind="ExternalInput").ap()
    wqt_d = nc.dram_tensor("wqt", [128, EC * FH], BF16,
                           kind="ExternalInput").ap()
    wkt_d = nc.dram_tensor("wkt", [128, EC * FH], BF16,
                           kind="ExternalInput").ap()
    wvt_d = nc.dram_tensor("wvt", [128, EC * FH], BF16,
                           kind="ExternalInput").ap()
    wot_d = nc.dram_tensor("wot", [128, FT * E], BF16,
                           kind="ExternalInput").ap()
    ones_d = nc.dram_tensor("ones", [128, DK], BF16, kind="ExternalInput").ap()
    y_d = nc.dram_tensor("y", [S, E], F32, kind="ExternalOutput").ap()

    with tile.TileContext(nc) as tc:
        with (
            tc.tile_pool(name="constp", bufs=1) as constp,
            tc.tile_pool(name="xtp", bufs=SC) as xtp,
            tc.tile_pool(name="wp", bufs=1) as wp,
            tc.tile_pool(name="qkp", bufs=2 * FT * SC) as qkp,
            tc.tile_pool(name="vp", bufs=ST) as vp,
            tc.tile_pool(name="cp", bufs=3) as cp,
            tc.tile_pool(name="ep", bufs=4) as ep,
            tc.tile_pool(name="aup", bufs=4) as aup,
            tc.tile_pool(name="smp", bufs=4) as smp,
            tc.tile_pool(name="op", bufs=3) as op,
            tc.tile_pool(name="o0p", bufs=8) as o0p,
            tc.tile_pool(name="mmp", bufs=2, space="PSUM") as mmp,
            tc.tile_pool(name="scp", bufs=2, space="PSUM") as scp,
            tc.tile_pool(name="atp", bufs=1, space="PSUM") as atp,
        ):
            # ---- preamble + input DMAs ----
            # first-needed first, split across three queues: gpsimd gets x0
            # and Wk (gate the first k-projection), scalar gets Wq and x1,
            # sync gets the rest in need order.
            warm = constp.tile([128, 512], BF16, tag="warm")
            nc.vector.memset(warm[:], 1.0)
            bias_t = constp.tile([128, 1], F32, tag="bias")
            nc.vector.memset(bias_t[:], EXP_BIAS)

            WQ = wp.tile([128, EC, FH], BF16, tag="wq")
            WK = wp.tile([128, EC, FH], BF16, tag="wk")
            WV = wp.tile([128, EC, FH], BF16, tag="wv")
            WO = wp.tile([128, FT, E], BF16, tag="wo")
            ones = constp.tile([128, DK], BF16, tag="ones")
            XSC = [xtp.tile([128, EC, 512], BF16, tag="xt", name=f"xt_{sc}")
                   for sc in range(SC)]

            # HBM bandwidth is fair-shared across queues at packet
            # granularity, so "priority" only exists WITHIN one queue:
            # serialize every input on the gpsimd queue in need order, each
            # transfer then gets the full ~300GB/s.
            nc.gpsimd.dma_start(XSC[0][:, 0:EC // 2, :],
                                xt_d[0][:, 0:EC * 256])
            nc.gpsimd.dma_start(WK[:], wkt_d)
            # Wq before x0b: the first q-projection half only needs x0a+Wq,
            # so it interleaves into the arrival ladder ~2us sooner and the
            # first exp starts ~3us earlier.
            nc.gpsimd.dma_start(WQ[:], wqt_d)
            nc.gpsimd.dma_start(XSC[0][:, EC // 2:EC, :],
                                xt_d[0][:, EC * 256:EC * 512])
            nc.gpsimd.dma_start(WV[:], wvt_d)
            nc.gpsimd.dma_start(XSC[1][:], xt_d[1])
            nc.gpsimd.dma_start(XSC[2][:], xt_d[2])
            nc.gpsimd.dma_start(XSC[3][:], xt_d[3])
            nc.gpsimd.dma_start(WO[:], wot_d)
            nc.sync.dma_start(ones[:], ones_d)
            # force the ACT exp-table DMA (~1.3us) now, not at the first
            # real score tile.
            dummy = constp.tile([128, 1], F32, tag="dummy")
            nc.scalar.activation(dummy[:], bias_t[:],
                                 mybir.ActivationFunctionType.Exp)
            onescol = constp.tile([128, HLOC], BF16, tag="onescol")
            nc.vector.memset(onescol[:], 1.0)

            # ---- PE warmup while the first operands stream in ----
            # enough matmuls to keep the HAM clock gate fed until x0/Wk land
            # (~5us); a cold gap here re-throttles the whole projection crunch.
            ps_w = mmp.tile([128, 512], F32, tag="mm", name="warmps")
            for _ in range(10):
                nc.tensor.matmul(ps_w[:, :], warm[:, 0:128], warm[:, :],
                                 start=True, stop=True)

            # ---- projections ----
            QTs = {}
            KTs = {}
            _half_state = {}

            def qk_proj_half(store, w, ft, sc, half):
                """Half of an 8-chunk projection chain (~1us of PE)."""
                if half == 0:
                    ps = mmp.tile([128, 512], F32, tag="mm", name="qkps")
                    _half_state[(id(store), ft, sc)] = ps
                else:
                    ps = _half_state.pop((id(store), ft, sc))
                for ec in range(half * 4, half * 4 + 4):
                    nc.tensor.matmul(
                        ps[:, :],
                        w[:, ec, ft * 128:(ft + 1) * 128],
                        XSC[sc][:, ec, :],
                        start=(ec == 0), stop=(ec == EC - 1),
                    )
                if half == 1:
                    dst = qkp.tile([128, 512], BF16, tag="qk",
                                   name=f"qk_{ft}_{sc}_{len(store)}")
                    nc.vector.tensor_copy(dst[:], ps[:, :])
                    store[(ft, sc)] = dst

            VAUG = [None] * ST

            def v_proj(jt):
                va = vp.tile([128, HLOC, DK + 1], BF16, tag="vaug")
                nc.vector.tensor_copy(va[:, :, DK:DK + 1],
                                      onescol[:, :, None])
                ps = mmp.tile([128, 512], F32, tag="mm", name="vps")
                for ec in range(EC):
                    nc.tensor.matmul(
                        ps[:, 0:FH],
                        XSC[jt // 4][:, ec, (jt % 4) * 128:(jt % 4 + 1) * 128],
                        WV[:, ec, :],
                        start=(ec == 0), stop=(ec == EC - 1),
                    )
                nc.vector.tensor_copy(
                    va[:, :, 0:DK],
                    ps[:, 0:FH].rearrange("p (h d) -> p h d", d=DK))
                VAUG[jt] = va

            # lead-in: the chunk-0 k/q for head pair 0, interleaved to
            # match the DMA arrival ladder (x0a, Wk, Wq, x0b).  Junk warm
            # matmuls (into the still-unused scores PSUM) fill the ladder
            # gaps so the HAM clock gate never re-throttles and the chains
            # run at full rate.
            ps_j = scp.tile([128, 512], F32, tag="sc", name="leadwarm")

            def _jw(n):
                for _ in range(n):
                    nc.tensor.matmul(ps_j[:, :], warm[:, 0:128], warm[:, :],
                                     start=True, stop=True)

            qk_proj_half(KTs, WK, 0, 0, 0)
            _jw(4)
            qk_proj_half(QTs, WQ, 0, 0, 0)
            _jw(3)
            qk_proj_half(KTs, WK, 0, 0, 1)
            qk_proj_half(QTs, WQ, 0, 0, 1)

            # ---- deadline-stamped work queue ----
            # pops() runs once per attention iteration: items whose deadline
            # is near always pop; otherwise at most one big PE item every few
            # iterations plus cheap ones, bounded on BOTH the PE and DVE
            # engines so interleaved work never stalls the exp chain.
            it_no = 0
            last_big = [-10]
            sched = []

            def push(fn, deadline, cost, dve=0, ready=0):
                sched.append([deadline, cost, dve, fn, ready])

            def pops():
                pe = 0
                dve = 0
                i = 0
                while i < len(sched) and pe < 900:
                    deadline, cost, dcost, fn, ready = sched[i]
                    if ready > it_no:
                        i += 1
                        continue
                    urgent = deadline - it_no <= 4
                    big = cost > 600
                    ok = urgent or (
                        pe + cost <= 900 and dve + dcost <= 1000 and
                        (not big or (pe == 0 and it_no >= 16 and
                                     it_no - last_big[0] >= 5)))
                    if ok:
                        sched.pop(i)
                        fn()
                        pe += max(cost, 200)
                        dve += dcost
                        if big:
                            last_big[0] = it_no
                    else:
                        i += 1

            def _qk(st, w, ft, sc, h):
                return lambda: qk_proj_half(st, w, ft, sc, h)

            # eager per-iteration emissions: stream order (not timing) is
            # what guarantees operands exist before their consumers.
            # v(jt) by iteration jt, KT(0,sc) by iteration 4sc-1,
            # KT(1,0)/QT(1,0) by iteration 15.
            eager = {i: [lambda jt=i: v_proj(jt)] for i in range(ST)}
            eager[2].append(_qk(KTs, WK, 0, 1, 0))
            eager[3].append(_qk(KTs, WK, 0, 1, 1))
            eager[6].append(_qk(KTs, WK, 0, 2, 0))
            eager[7].append(_qk(KTs, WK, 0, 2, 1))
            eager[10].append(_qk(KTs, WK, 0, 3, 0))
            eager[11].append(_qk(KTs, WK, 0, 3, 1))
            eager[12].append(_qk(KTs, WK, 1, 0, 0))
            eager[13].append(_qk(KTs, WK, 1, 0, 1))
            eager[14].append(_qk(QTs, WQ, 1, 0, 0))
            eager[15].append(_qk(QTs, WQ, 1, 0, 1))

            # remaining projections with their use-iteration deadlines.
            for _ft, _sc, _dl in (
                    (1, 1, 20), (1, 2, 24), (1, 3, 28)):
                for _h in range(2):
                    push(_qk(KTs, WK, _ft, _sc, _h), _dl - 1 + _h, 1050)
            for _ft, _sc, _dl in (
                    (0, 1, 26), (1, 1, 38), (0, 2, 54),
                    (1, 2, 70), (0, 3, 86), (1, 3, 102)):
                for _h in range(2):
                    push(_qk(QTs, WQ, _ft, _sc, _h), _dl + _h, 1050)

            # ---- per-chunk epilogue builders ----
            def make_normalize(ic, concat, au_fts, dn):
                rd = smp.tile([128, 512], BF16, tag="rd")

                def recip(rd=rd, dn=dn):
                    with nc.allow_low_precision(
                            reason="bf16 softmax denominators"):
                        nc.vector.reciprocal(rd[:], dn[:])
                push(recip, it_no + 3, 0, dve=1000)

                for h in range(HLOC):
                    def norm_h(h=h, rd=rd, concat=concat,
                               au_fts=tuple(au_fts)):
                        ft, hs = h // 2, h % 2
                        pb = hs * DK
                        ps_b = mmp.tile([DK, 512], F32, tag="mm", name="bc")
                        nc.tensor.matmul(ps_b[:, :],
                                         ones[h * 32:h * 32 + 1, :],
                                         rd[h * 32:h * 32 + 1, :],
                                         start=True, stop=True,
                                         tile_position=(h * 32, 0))
                        with nc.allow_low_precision(
                                reason="bf16 attn concat"):
                            nc.vector.tensor_tensor(
                                concat[pb:pb + DK, ft, :],
                                au_fts[ft][0:DK, hs * 512:(hs + 1) * 512],
                                ps_b[:, :], mybir.AluOpType.mult)
                    push(norm_h, it_no + 14 + 2 * h, 300, dve=700,
                         ready=it_no + 12 + h)

            # per-head-pair variant for the last chunk's tail.  The ft=1
            # reciprocal runs AFTER the final exp, so it can use the scalar
            # engine's table-based reciprocal (one table switch, ~4x faster
            # than DVE and off the DVE critical path).
            def make_normalize_ft(ic, ft, concat, au, dn):
                rd = smp.tile([128, 512], BF16, tag="rd")

                def recip(rd=rd, dn=dn):
                    with nc.allow_low_precision(
                            reason="bf16 softmax denominators"):
                        nc.vector.reciprocal(rd[:], dn[:])
                push(recip, it_no + 3, 0, dve=1000)

                for hs in range(2):
                    def norm_h(ft=ft, hs=hs, rd=rd, concat=concat, au=au):
                        h = ft * 2 + hs
                        pb = hs * DK
                        ps_b = mmp.tile([DK, 512], F32, tag="mm", name="bc")
                        nc.tensor.matmul(ps_b[:, :],
                                         ones[h * 32:h * 32 + 1, :],
                                         rd[h * 32:h * 32 + 1, :],
                                         start=True, stop=True,
                                         tile_position=(h * 32, 0))
                        with nc.allow_low_precision(
                                reason="bf16 attn concat"):
                            nc.vector.tensor_tensor(
                                concat[pb:pb + DK, ft, :],
                                au[0:DK, hs * 512:(hs + 1) * 512],
                                ps_b[:, :], mybir.AluOpType.mult)
                    push(norm_h, it_no + 12 + hs, 300, dve=700,
                         ready=it_no + 10 + hs)

            def make_phase_c(ic, concat):
                for stl in range(4):
                    st = ic * 4 + stl
                    for oc in range(2):
                        def emit(st=st, oc=oc, stl=stl, concat=concat):
                            ps_o = mmp.tile([128, 512], F32, tag="mm",
                                            name="ops")
                            for fc in range(FT):
                                nc.tensor.matmul(
                                    ps_o[:, :],
                                    concat[:, fc, stl * 128:(stl + 1) * 128],
                                    WO[:, fc, oc * 512:(oc + 1) * 512],
                                    start=(fc == 0), stop=(fc == FT - 1),
                                )
                            ot = op.tile([128, 512], F32, tag="out")
                            nc.vector.tensor_copy(ot[:], ps_o[:, :])
                            nc.gpsimd.dma_start(
                                y_d[st * 128:(st + 1) * 128,
                                    oc * 512:(oc + 1) * 512],
                                ot[:])
                        push(emit, it_no + 22 + 2 * stl + oc, 520, dve=700,
                             ready=it_no + 18 + stl)

            # the LAST chunk's output projection is split by head pair so its
            # first half runs during the final 16 attention iterations and
            # the drain tail only carries one matmul + add per output tile.
            _ot0 = {}

            def make_phase_c_a(ic, concat):
                for stl in range(4):
                    for oc in range(2):
                        def emit_a(oc=oc, stl=stl, concat=concat):
                            ps_o = mmp.tile([128, 512], F32, tag="mm",
                                            name="opsa")
                            nc.tensor.matmul(
                                ps_o[:, :],
                                concat[:, 0, stl * 128:(stl + 1) * 128],
                                WO[:, 0, oc * 512:(oc + 1) * 512],
                                start=True, stop=True)
                            ot0 = o0p.tile([128, 512], F32, tag="out0")
                            nc.vector.tensor_copy(ot0[:], ps_o[:, :])
                            _ot0[(stl, oc)] = ot0
                        push(emit_a, it_no + 15 + 2 * stl + oc, 450, dve=700,
                             ready=it_no + 13 + stl)

            def make_phase_c_b(ic, concat):
                for stl in range(4):
                    st = ic * 4 + stl
                    for oc in range(2):
                        def emit_b(st=st, oc=oc, stl=stl, concat=concat):
                            ps_o = mmp.tile([128, 512], F32, tag="mm",
                                            name="opsb")
                            nc.tensor.matmul(
                                ps_o[:, :],
                                concat[:, 1, stl * 128:(stl + 1) * 128],
                                WO[:, 1, oc * 512:(oc + 1) * 512],
                                start=True, stop=True)
                            ot = op.tile([128, 512], F32, tag="out")
                            nc.vector.tensor_tensor(
                                ot[:], _ot0[(stl, oc)][:], ps_o[:, :],
                                mybir.AluOpType.add)
                            nc.gpsimd.dma_start(
                                y_d[st * 128:(st + 1) * 128,
                                    oc * 512:(oc + 1) * 512],
                                ot[:])
                        push(emit_b, it_no + 14 + 2 * stl + oc, 450, dve=700,
                             ready=it_no + 12 + stl)

            # ---- attention: software-pipelined over (ic, ft, jt) ----
            DNS = []
            for _ic in range(SC):
                _dn = smp.tile([128, 512], F32, tag="dn", name=f"dn_{_ic}")
                nc.vector.memset(_dn[:], 1.0)  # unused lanes stay finite
                DNS.append(_dn)
            prev_pv = None      # PV matmuls of the previous iteration
            post_pv = None      # runs right after the PV that closes a pair
            for ic in range(SC):
                concat = cp.tile([128, FT, 512], BF16, tag="concat")
                dn = DNS[ic]
                au_fts = []
                for ft in range(FT):
                    ps_ap = atp.tile([128, 1024], F32, tag="at")
                    for jt in range(ST):
                        ps_s = scp.tile([128, 1024], F32, tag="sc")
                        for hs in range(2):
                            pb = hs * DK
                            nc.tensor.matmul(
                                ps_s[:, hs * 512:(hs + 1) * 512],
                                KTs[(ft, jt // 4)][pb:pb + DK,
                                                   (jt % 4) * 128:
                                                   (jt % 4 + 1) * 128],
                                QTs[(ft, ic)][pb:pb + DK, :],
                                start=True, stop=True,
                            )
                        ex = ep.tile([128, 1024], BF16, tag="exp")
                        with nc.allow_low_precision(reason="bf16 softmax"):
                            nc.scalar.activation(
                                ex[:], ps_s[:],
                                mybir.ActivationFunctionType.Exp,
                                bias=bias_t[:], scale=1.0 / np.sqrt(DK))

                        if prev_pv is not None:
                            prev_pv()
                        if post_pv is not None:
                            post_pv()
                            post_pv = None

                        def pv(ps_ap=ps_ap, ex=ex, ft=ft, jt=jt):
                            for hs in range(2):
                                nc.tensor.matmul(
                                    ps_ap[0:DK + 1,
                                          hs * 512:(hs + 1) * 512],
                                    VAUG[jt][:, ft * 2 + hs, :],
                                    ex[:, hs * 512:(hs + 1) * 512],
                                    start=(jt == 0), stop=(jt == ST - 1),
                                )
                        prev_pv = pv

                        if jt == ST - 1:
                            def drain(ic=ic, ft=ft, ps_ap=ps_ap, dn=dn,
                                      concat=concat, au_fts=au_fts):
                                # one copy frees the PV PSUM bank; row 64
                                # holds the softmax denominators.
                                au = aup.tile([DK + 1, 1024], F32, tag="au")
                                last = ic == SC - 1 and ft == FT - 1
                                if last:
                                    # exp chain is done: the scalar engine is
                                    # idle, so pull the denominator rows AND
                                    # the attention rows off PSUM there; the
                                    # DVE goes straight to the reciprocal.
                                    for hs in range(2):
                                        dpb = (ft * 2 + hs) * 32
                                        nc.scalar.copy(
                                            dn[dpb:dpb + 1, :],
                                            ps_ap[DK:DK + 1,
                                                  hs * 512:(hs + 1) * 512])
                                    nc.scalar.copy(au[:],
                                                   ps_ap[0:DK + 1, :])
                                else:
                                    nc.vector.tensor_copy(au[:],
                                                          ps_ap[0:DK + 1, :])
                                au_fts.append(au)
                                if not last:
                                    for hs in range(2):
                                        dpb = (ft * 2 + hs) * 32
                                        nc.vector.tensor_copy(
                                            dn[dpb:dpb + 1, :],
                                            au[DK:DK + 1,
                                               hs * 512:(hs + 1) * 512])
                                if ft == FT - 1:
                                    make_normalize(ic, concat, au_fts, dn)
                                    make_phase_c(ic, concat)
                            post_pv = drain

                        for fn in eager.pop(it_no, []):
                            fn()
                        pops()
                        it_no += 1

            prev_pv()
            post_pv()
            # keep the PE warm through the reciprocal/normalize tail so the
            # final output-projection matmuls run at full clock.
            # sized to the scalar-assisted drain: the reciprocal now ends
            # ~4.9us after the last PV, so 12 keepers (~4.6us) bridge it
            # without delaying the normalize matmuls behind them.
            ps_wt = scp.tile([128, 512], F32, tag="sc", name="tailwarm")
            for _ in range(12):
                nc.tensor.matmul(ps_wt[:, 0:512], warm[:, 0:128], warm[:, :],
                                 start=True, stop=True)
            while sched:
                it_no += 1
                before = len(sched)
                pops()
                if len(sched) == before:
                    sched[0][0] = it_no  # force the oldest item ripe
                    sched[0][4] = it_no

    nc.compile()
    return nc


_PROGRAM = None


def _get_program() -> bass.Bass:
    global _PROGRAM
    if _PROGRAM is None:
        _PROGRAM = _build_program()
    return _PROGRAM


def _bf16(a: np.ndarray) -> np.ndarray:
    return np.ascontiguousarray(a).astype(ml_dtypes.bfloat16)


def _pack_w(wt: np.ndarray, cols: int) -> np.ndarray:
    """[E_contract, cols] -> SBUF tile layout [128, EC_chunks * cols]."""
    k = wt.shape[0]
    return np.ascontiguousarray(
        wt.reshape(k // 128, 128, cols).transpose(1, 0, 2).reshape(128, -1))


def _prepare_in_maps(x, Wq, Wk, Wv, Wo):
    x = np.asarray(x, dtype=np.float32)
    Wq = np.asarray(Wq, dtype=np.float32)
    Wk = np.asarray(Wk, dtype=np.float32)
    Wv = np.asarray(Wv, dtype=np.float32)
    Wo = np.asarray(Wo, dtype=np.float32)
    in_maps = []
    for c in range(NCORES):
        b, hg = c // HGROUPS, c % HGROUPS
        rows = slice(hg * FH, (hg + 1) * FH)
        xt = x[b].T  # [E, S]
        # [SC, 128, EC*512]: per s-chunk, the exact SBUF tile layout.
        xt_tiled = (xt.reshape(EC, 128, SC, 512).transpose(2, 1, 0, 3)
                    .reshape(SC, 128, EC * 512))
        in_maps.append({
            "xt": _bf16(xt_tiled),
            "wqt": _bf16(_pack_w(Wq[rows, :].T, FH)),
            "wkt": _bf16(_pack_w(Wk[rows, :].T, FH)),
            "wvt": _bf16(_pack_w(Wv[rows, :].T, FH)),
            "wot": _bf16(_pack_w(Wo[:, rows].T, E)),
            "ones": np.ones((128, DK), ml_dtypes.bfloat16),
        })
    return in_maps


def run(inputs: dict, **spmd_kwargs):
    """Run on all 8 cores; returns (full output, BassKernelResults)."""
    nc = _get_program()
    in_maps = _prepare_in_maps(**inputs)
    res = run_bass_kernel_spmd(nc, in_maps, core_ids=list(range(NCORES)),
                               **spmd_kwargs)
    partials = [r["y"] for r in res.results]
    out = np.empty((B, S, E), dtype=np.float32)
    for b in range(B):
        acc = partials[b * HGROUPS].astype(np.float32, copy=True)
        for hg in range(1, HGROUPS):
            acc += partials[b * HGROUPS + hg]
        out[b] = acc
    return out, res


def kernel(**inputs) -> np.ndarray:
    out, _ = run(inputs)
    return out


# revision 36
# speedup vs baseline: 1.0145x; 1.0145x over previous
"""Multi-head attention (B=2, S=2048, E=1024, H=16) on 8 Trainium2 NeuronCores.

Sharding: data-parallel over the 2 batches x tensor-parallel over 4 head-groups
(4 heads each).  Core c handles batch c//4, heads [4*(c%4), 4*(c%4)+4).
Each core computes its heads' Q/K/V projections, softmax(QK^T/8)V, and the
partial output projection against the matching Wo column slice; the host sums
the 4 partials per batch (the head-parallel all-reduce) and stacks batches.

Device-side design (bf16 everywhere, fp32 PSUM accumulate):
 - The kernel is scalar-engine bound: 128 exp tiles of [128,1024] at ~1.15us
   each is ~147us that nothing else can absorb (DVE has no exp).  Everything
   is scheduled so the ACT chain never stalls:
   * scores for iteration n+1 issue on the PE BEFORE the PV matmuls of
     iteration n (PV waits on ACT(n), scores don't), so the in-order PE
     stream can't starve the scalar engine.
   * projection/epilogue work is doled out from a deadline-stamped queue at
     a bounded rate (one ~1us chain-half only when its deadline nears or
     every few iterations) so iterations stay under the ACT period.
 - Host pre-packs every dram tensor in its exact SBUF tile layout, so each
   input DMA is one contiguous descriptor per partition (~0.7us issue,
   full bandwidth), and the first-needed tensors are issued first across
   three different engine queues.
 - Scores are produced transposed, sT[j, i] = k_j . q_i; head pairs sit in
   PE row groups 0-63/64-127 so their K=64 score matmuls run concurrently.
   softmax's partition-dim sum rides the PV matmul as a ones column on V.
 - The attention PSUM bank is drained by ONE [65,1024] copy per head pair;
   the softmax reciprocal (slow on DVE) is stamped a few iterations later so
   its dependent broadcast-matmul never blocks the PE stream.
"""

import numpy as np
import ml_dtypes

import concourse.bass as bass
from concourse import bacc
import concourse.mybir as mybir
import concourse.tile as tile
from concourse.bass_utils import run_bass_kernel_spmd

B, S, E, H = 2, 2048, 1024, 16
DK = 64
NCORES = 8
HGROUPS = 4            # head-parallel groups per batch
HLOC = H // HGROUPS    # heads per core = 4
FH = HLOC * DK         # local feature cols = 256

EC = E // 128        # 8 contraction chunks for the projections
ST = S // 128        # 16 seq tiles of 128 (the j tiles)
SC = S // 512        # 4 seq chunks of 512 (the i chunks)
FT = FH // 128       # 2 feature tiles (head pairs)

F32 = mybir.dt.float32
BF16 = mybir.dt.bfloat16
EXP_BIAS = -4.0        # constant shift inside exp; cancels in softmax


def _build_program() -> bass.Bass:
    nc = bacc.Bacc("TRN2", target_bir_lowering=False, debug=False,
                   enable_asserts=False)

    # all inputs are pre-packed on the host into the exact SBUF tile layout,
    # so every DMA below is contiguous.
    xt_d = nc.dram_tensor("xt", [SC, 128, EC * 512], BF16,
                          kind="ExternalInput").ap()
    wqt_d = nc.dram_tensor("wqt", [128, EC * FH], BF16,
                           kind="ExternalInput").ap()
    wkt_d = nc.dram_tensor("wkt", [128, EC * FH], BF16,
                           kind="ExternalInput").ap()
    wvt_d = nc.dram_tensor("wvt", [128, EC * FH], BF16,
                           kind="ExternalInput").ap()
    wot_d = nc.dram_tensor("wot", [128, FT * E], BF16,
                           kind="ExternalInput").ap()
    ones_d = nc.dram_tensor("ones", [128, DK], BF16, kind="ExternalInput").ap()
    y_d = nc.dram_tensor("y", [S, E], F32, kind="ExternalOutput").ap()

    with tile.TileContext(nc) as tc:
        with (
            tc.tile_pool(name="constp", bufs=1) as constp,
            tc.tile_pool(name="xtp", bufs=SC) as xtp,
            tc.tile_pool(name="wp", bufs=1) as wp,
            tc.tile_pool(name="qkp", bufs=2 * FT * SC) as qkp,
            tc.tile_pool(name="vp", bufs=ST) as vp,
            tc.tile_pool(name="cp", bufs=3) as cp,
            tc.tile_pool(name="ep", bufs=4) as ep,
            tc.tile_pool(name="aup", bufs=4) as aup,
            tc.tile_pool(name="smp", bufs=4) as smp,
            tc.tile_pool(name="op", bufs=3) as op,
            tc.tile_pool(name="o0p", bufs=8) as o0p,
            tc.tile_pool(name="mmp", bufs=2, space="PSUM") as mmp,
            tc.tile_pool(name="scp", bufs=2, space="PSUM") as scp,
            tc.tile_pool(name="atp", bufs=1, space="PSUM") as atp,
        ):
            # ---- preamble + input DMAs ----
            # first-needed first, split across three queues: gpsimd gets x0
            # and Wk (gate the first k-projection), scalar gets Wq and x1,
            # sync gets the rest in need order.
            warm = constp.tile([128, 512], BF16, tag="warm")
            nc.vector.memset(warm[:], 1.0)
            bias_t = constp.tile([128, 1], F32, tag="bias")
            nc.vector.memset(bias_t[:], EXP_BIAS)

            WQ = wp.tile([128, EC, FH], BF16, tag="wq")
            WK = wp.tile([128, EC, FH], BF16, tag="wk")
            WV = wp.tile([128, EC, FH], BF16, tag="wv")
            WO = wp.tile([128, FT, E], BF16, tag="wo")
            ones = constp.tile([128, DK], BF16, tag="ones")
            XSC = [xtp.tile([128, EC, 512], BF16, tag="xt", name=f"xt_{sc}")
                   for sc in range(SC)]

            # HBM bandwidth is fair-shared across queues at packet
            # granularity, so "priority" only exists WITHIN one queue:
            # serialize every input on the gpsimd queue in need order, each
            # transfer then gets the full ~300GB/s.
            nc.gpsimd.dma_start(XSC[0][:, 0:EC // 2, :],
                                xt_d[0][:, 0:EC * 256])
            nc.gpsimd.dma_start(WK[:], wkt_d)
            # Wq before x0b: the first q-projection half only needs x0a+Wq,
            # so it interleaves into the arrival ladder ~2us sooner and the
            # first exp starts ~3us earlier.
            nc.gpsimd.dma_start(WQ[:], wqt_d)
            nc.gpsimd.dma_start(XSC[0][:, EC // 2:EC, :],
                                xt_d[0][:, EC * 256:EC * 512])
            nc.gpsimd.dma_start(WV[:], wvt_d)
            nc.gpsimd.dma_start(XSC[1][:], xt_d[1])
            nc.gpsimd.dma_start(XSC[2][:], xt_d[2])
            nc.gpsimd.dma_start(XSC[3][:], xt_d[3])
            nc.gpsimd.dma_start(WO[:], wot_d)
            nc.sync.dma_start(ones[:], ones_d)
            # force the ACT exp-table DMA (~1.3us) now, not at the first
            # real score tile.
            dummy = constp.tile([128, 1], F32, tag="dummy")
            nc.scalar.activation(dummy[:], bias_t[:],
                                 mybir.ActivationFunctionType.Exp)
            onescol = constp.tile([128, HLOC], BF16, tag="onescol")
            nc.vector.memset(onescol[:], 1.0)

            # ---- PE warmup while the first operands stream in ----
            # enough matmuls to keep the HAM clock gate fed until x0/Wk land
            # (~5us); a cold gap here re-throttles the whole projection crunch.
            ps_w = mmp.tile([128, 512], F32, tag="mm", name="warmps")
            for _ in range(10):
                nc.tensor.matmul(ps_w[:, :], warm[:, 0:128], warm[:, :],
                                 start=True, stop=True)

            # ---- projections ----
            QTs = {}
            KTs = {}
            _half_state = {}

            def qk_proj_half(store, w, ft, sc, half):
                """Half of an 8-chunk projection chain (~1us of PE)."""
                if half == 0:
                    ps = mmp.tile([128, 512], F32, tag="mm", name="qkps")
                    _half_state[(id(store), ft, sc)] = ps
                else:
                    ps = _half_state.pop((id(store), ft, sc))
                for ec in range(half * 4, half * 4 + 4):
                    nc.tensor.matmul(
                        ps[:, :],
                        w[:, ec, ft * 128:(ft + 1) * 128],
                        XSC[sc][:, ec, :],
                        start=(ec == 0), stop=(ec == EC - 1),
                    )
                if half == 1:
                    dst = qkp.tile([128, 512], BF16, tag="qk",
                                   name=f"qk_{ft}_{sc}_{len(store)}")
                    nc.vector.tensor_copy(dst[:], ps[:, :])
                    store[(ft, sc)] = dst

            VAUG = [None] * ST

            def v_proj(jt):
                va = vp.tile([128, HLOC, DK + 1], BF16, tag="vaug")
                nc.vector.tensor_copy(va[:, :, DK:DK + 1],
                                      onescol[:, :, None])
                ps = mmp.tile([128, 512], F32, tag="mm", name="vps")
                for ec in range(EC):
                    nc.tensor.matmul(
                        ps[:, 0:FH],
                        XSC[jt // 4][:, ec, (jt % 4) * 128:(jt % 4 + 1) * 128],
                        WV[:, ec, :],
                        start=(ec == 0), stop=(ec == EC - 1),
                    )
                nc.vector.tensor_copy(
                    va[:, :, 0:DK],
                    ps[:, 0:FH].rearrange("p (h d) -> p h d", d=DK))
                VAUG[jt] = va

            # lead-in: the chunk-0 k/q for head pair 0, interleaved to
            # match the DMA arrival ladder (x0a, Wk, Wq, x0b).  Junk warm
            # matmuls (into the still-unused scores PSUM) fill the ladder
            # gaps so the HAM clock gate never re-throttles and the chains
            # run at full rate.
            ps_j = scp.tile([128, 512], F32, tag="sc", name="leadwarm")

            def _jw(n):
                for _ in range(n):
                    nc.tensor.matmul(ps_j[:, :], warm[:, 0:128], warm[:, :],
                                     start=True, stop=True)

            qk_proj_half(KTs, WK, 0, 0, 0)
            _jw(4)
            qk_proj_half(QTs, WQ, 0, 0, 0)
            _jw(3)
            qk_proj_half(KTs, WK, 0, 0, 1)
            qk_proj_half(QTs, WQ, 0, 0, 1)

            # ---- deadline-stamped work queue ----
            # pops() runs once per attention iteration: items whose deadline
            # is near always pop; otherwise at most one big PE item every few
            # iterations plus cheap ones, bounded on BOTH the PE and DVE
            # engines so interleaved work never stalls the exp chain.
            it_no = 0
            last_big = [-10]
            sched = []

            def push(fn, deadline, cost, dve=0, ready=0):
                sched.append([deadline, cost, dve, fn, ready])

            def pops():
                pe = 0
                dve = 0
                i = 0
                while i < len(sched) and pe < 900:
                    deadline, cost, dcost, fn, ready = sched[i]
                    if ready > it_no:
                        i += 1
                        continue
                    urgent = deadline - it_no <= 4
                    big = cost > 600
                    ok = urgent or (
                        pe + cost <= 900 and dve + dcost <= 1000 and
                        (not big or (pe == 0 and it_no >= 16 and
                                     it_no - last_big[0] >= 5)))
                    if ok:
                        sched.pop(i)
                        fn()
                        pe += max(cost, 200)
                        dve += dcost
                        if big:
                            last_big[0] = it_no
                    else:
                        i += 1

            def _qk(st, w, ft, sc, h):
                return lambda: qk_proj_half(st, w, ft, sc, h)

            # eager per-iteration emissions: stream order (not timing) is
            # what guarantees operands exist before their consumers.
            # v(jt) by iteration jt, KT(0,sc) by iteration 4sc-1,
            # KT(1,0)/QT(1,0) by iteration 15.
            eager = {i: [lambda jt=i: v_proj(jt)] for i in range(ST)}
            eager[2].append(_qk(KTs, WK, 0, 1, 0))
            eager[3].append(_qk(KTs, WK, 0, 1, 1))
            eager[6].append(_qk(KTs, WK, 0, 2, 0))
            eager[7].append(_qk(KTs, WK, 0, 2, 1))
            eager[10].append(_qk(KTs, WK, 0, 3, 0))
            eager[11].append(_qk(KTs, WK, 0, 3, 1))
            eager[12].append(_qk(KTs, WK, 1, 0, 0))
            eager[13].append(_qk(KTs, WK, 1, 0, 1))
            eager[14].append(_qk(QTs, WQ, 1, 0, 0))
            eager[15].append(_qk(QTs, WQ, 1, 0, 1))

            # remaining projections with their use-iteration deadlines.
            for _ft, _sc, _dl in (
                    (1, 1, 20), (1, 2, 24), (1, 3, 28)):
                for _h in range(2):
                    push(_qk(KTs, WK, _ft, _sc, _h), _dl - 1 + _h, 1050)
            for _ft, _sc, _dl in (
                    (0, 1, 26), (1, 1, 38), (0, 2, 54),
                    (1, 2, 70), (0, 3, 86), (1, 3, 102)):
                for _h in range(2):
                    push(_qk(QTs, WQ, _ft, _sc, _h), _dl + _h, 1050)

            # ---- per-chunk epilogue builders ----
            def make_normalize(ic, concat, au_fts, dn):
                rd = smp.tile([128, 512], BF16, tag="rd")

                def recip(rd=rd, dn=dn):
                    with nc.allow_low_precision(
                            reason="bf16 softmax denominators"):
                        nc.vector.reciprocal(rd[:], dn[:])
                push(recip, it_no + 3, 0, dve=1000)

                for h in range(HLOC):
                    def norm_h(h=h, rd=rd, concat=concat,
                               au_fts=tuple(au_fts)):
                        ft, hs = h // 2, h % 2
                        pb = hs * DK
                        ps_b = mmp.tile([DK, 512], F32, tag="mm", name="bc")
                        nc.tensor.matmul(ps_b[:, :],
                                         ones[h * 32:h * 32 + 1, :],
                                         rd[h * 32:h * 32 + 1, :],
                                         start=True, stop=True,
                                         tile_position=(h * 32, 0))
                        with nc.allow_low_precision(
                                reason="bf16 attn concat"):
                            nc.vector.tensor_tensor(
                                concat[pb:pb + DK, ft, :],
                                au_fts[ft][0:DK, hs * 512:(hs + 1) * 512],
                                ps_b[:, :], mybir.AluOpType.mult)
                    push(norm_h, it_no + 14 + 2 * h, 300, dve=700,
                         ready=it_no + 12 + h)

            # per-head-pair variant for the last chunk's tail.  The ft=1
            # reciprocal runs AFTER the final exp, so it can use the scalar
            # engine's table-based reciprocal (one table switch, ~4x faster
            # than DVE and off the DVE critical path).
            def make_normalize_ft(ic, ft, concat, au, dn):
                rd = smp.tile([128, 512], BF16, tag="rd")

                def recip(rd=rd, dn=dn):
                    with nc.allow_low_precision(
                            reason="bf16 softmax denominators"):
                        nc.vector.reciprocal(rd[:], dn[:])
                push(recip, it_no + 3, 0, dve=1000)

                for hs in range(2):
                    def norm_h(ft=ft, hs=hs, rd=rd, concat=concat, au=au):
                        h = ft * 2 + hs
                        pb = hs * DK
                        ps_b = mmp.tile([DK, 512], F32, tag="mm", name="bc")
                        nc.tensor.matmul(ps_b[:, :],
                                         ones[h * 32:h * 32 + 1, :],
                                         rd[h * 32:h * 32 + 1, :],
                                         start=True, stop=True,
                                         tile_position=(h * 32, 0))
                        with nc.allow_low_precision(
                                reason="bf16 attn concat"):
                            nc.vector.tensor_tensor(
                                concat[pb:pb + DK, ft, :],
                                au[0:DK, hs * 512:(hs + 1) * 512],
                                ps_b[:, :], mybir.AluOpType.mult)
                    push(norm_h, it_no + 12 + hs, 300, dve=700,
                         ready=it_no + 10 + hs)

            def make_phase_c(ic, concat):
                for stl in range(4):
                    st = ic * 4 + stl
                    for oc in range(2):
                        def emit(st=st, oc=oc, stl=stl, concat=concat,
                                 ic=ic):
                            ps_o = mmp.tile([128, 512], F32, tag="mm",
                                            name="ops")
                            for fc in range(FT):
                                nc.tensor.matmul(
                                    ps_o[:, :],
                                    concat[:, fc, stl * 128:(stl + 1) * 128],
                                    WO[:, fc, oc * 512:(oc + 1) * 512],
                                    start=(fc == 0), stop=(fc == FT - 1),
                                )
                            ot = op.tile([128, 512], F32, tag="out")
                            if ic == SC - 1:
                                # tail: the exp chain is finished, so drain
                                # PSUM on the idle scalar engine and leave
                                # the DVE to the normalize chain.
                                nc.scalar.copy(ot[:], ps_o[:, :])
                            else:
                                nc.vector.tensor_copy(ot[:], ps_o[:, :])
                            nc.gpsimd.dma_start(
                                y_d[st * 128:(st + 1) * 128,
                                    oc * 512:(oc + 1) * 512],
                                ot[:])
                        push(emit, it_no + 22 + 2 * stl + oc, 520, dve=700,
                             ready=it_no + 18 + stl)

            # the LAST chunk's output projection is split by head pair so its
            # first half runs during the final 16 attention iterations and
            # the drain tail only carries one matmul + add per output tile.
            _ot0 = {}

            def make_phase_c_a(ic, concat):
                for stl in range(4):
                    for oc in range(2):
                        def emit_a(oc=oc, stl=stl, concat=concat):
                            ps_o = mmp.tile([128, 512], F32, tag="mm",
                                            name="opsa")
                            nc.tensor.matmul(
                                ps_o[:, :],
                                concat[:, 0, stl * 128:(stl + 1) * 128],
                                WO[:, 0, oc * 512:(oc + 1) * 512],
                                start=True, stop=True)
                            ot0 = o0p.tile([128, 512], F32, tag="out0")
                            nc.vector.tensor_copy(ot0[:], ps_o[:, :])
                            _ot0[(stl, oc)] = ot0
                        push(emit_a, it_no + 15 + 2 * stl + oc, 450, dve=700,
                             ready=it_no + 13 + stl)

            def make_phase_c_b(ic, concat):
                for stl in range(4):
                    st = ic * 4 + stl
                    for oc in range(2):
                        def emit_b(st=st, oc=oc, stl=stl, concat=concat):
                            ps_o = mmp.tile([128, 512], F32, tag="mm",
                                            name="opsb")
                            nc.tensor.matmul(
                                ps_o[:, :],
                                concat[:, 1, stl * 128:(stl + 1) * 128],
                                WO[:, 1, oc * 512:(oc + 1) * 512],
                                start=True, stop=True)
                            ot = op.tile([128, 512], F32, tag="out")
                            nc.vector.tensor_tensor(
                                ot[:], _ot0[(stl, oc)][:], ps_o[:, :],
                                mybir.AluOpType.add)
                            nc.gpsimd.dma_start(
                                y_d[st * 128:(st + 1) * 128,
                                    oc * 512:(oc + 1) * 512],
                                ot[:])
                        push(emit_b, it_no + 14 + 2 * stl + oc, 450, dve=700,
                             ready=it_no + 12 + stl)

            # ---- attention: software-pipelined over (ic, ft, jt) ----
            DNS = []
            for _ic in range(SC):
                _dn = smp.tile([128, 512], F32, tag="dn", name=f"dn_{_ic}")
                nc.vector.memset(_dn[:], 1.0)  # unused lanes stay finite
                DNS.append(_dn)
            prev_pv = None      # PV matmuls of the previous iteration
            post_pv = None      # runs right after the PV that closes a pair
            for ic in range(SC):
                concat = cp.tile([128, FT, 512], BF16, tag="concat")
                dn = DNS[ic]
                au_fts = []
                for ft in range(FT):
                    ps_ap = atp.tile([128, 1024], F32, tag="at")
                    for jt in range(ST):
                        ps_s = scp.tile([128, 1024], F32, tag="sc")
                        for hs in range(2):
                            pb = hs * DK
                            nc.tensor.matmul(
                                ps_s[:, hs * 512:(hs + 1) * 512],
                                KTs[(ft, jt // 4)][pb:pb + DK,
                                                   (jt % 4) * 128:
                                                   (jt % 4 + 1) * 128],
                                QTs[(ft, ic)][pb:pb + DK, :],
                                start=True, stop=True,
                            )
                        ex = ep.tile([128, 1024], BF16, tag="exp")
                        with nc.allow_low_precision(reason="bf16 softmax"):
                            nc.scalar.activation(
                                ex[:], ps_s[:],
                                mybir.ActivationFunctionType.Exp,
                                bias=bias_t[:], scale=1.0 / np.sqrt(DK))

                        if prev_pv is not None:
                            prev_pv()
                        if post_pv is not None:
                            post_pv()
                            post_pv = None

                        def pv(ps_ap=ps_ap, ex=ex, ft=ft, jt=jt):
                            for hs in range(2):
                                nc.tensor.matmul(
                                    ps_ap[0:DK + 1,
                                          hs * 512:(hs + 1) * 512],
                                    VAUG[jt][:, ft * 2 + hs, :],
                                    ex[:, hs * 512:(hs + 1) * 512],
                                    start=(jt == 0), stop=(jt == ST - 1),
                                )
                        prev_pv = pv

                        if jt == ST - 1:
                            def drain(ic=ic, ft=ft, ps_ap=ps_ap, dn=dn,
                                      concat=concat, au_fts=au_fts):
                                # one copy frees the PV PSUM bank; row 64
                                # holds the softmax denominators.
                                au = aup.tile([DK + 1, 1024], F32, tag="au")
                                last = ic == SC - 1 and ft == FT - 1
                                if last:
                                    # exp chain is done: the scalar engine is
                                    # idle, so pull the denominator rows AND
                                    # the attention rows off PSUM there; the
                                    # DVE goes straight to the reciprocal.
                                    for hs in range(2):
                                        dpb = (ft * 2 + hs) * 32
                                        nc.scalar.copy(
                                            dn[dpb:dpb + 1, :],
                                            ps_ap[DK:DK + 1,
                                                  hs * 512:(hs + 1) * 512])
                                    nc.scalar.copy(au[:],
                                                   ps_ap[0:DK + 1, :])
                                else:
                                    nc.vector.tensor_copy(au[:],
                                                          ps_ap[0:DK + 1, :])
                                au_fts.append(au)
                                if not last:
                                    for hs in range(2):
                                        dpb = (ft * 2 + hs) * 32
                                        nc.vector.tensor_copy(
                                            dn[dpb:dpb + 1, :],
                                            au[DK:DK + 1,
                                               hs * 512:(hs + 1) * 512])
                                if ft == FT - 1:
                                    make_normalize(ic, concat, au_fts, dn)
                                    make_phase_c(ic, concat)
                            post_pv = drain

                        for fn in eager.pop(it_no, []):
                            fn()
                        pops()
                        it_no += 1

            prev_pv()
            post_pv()
            # keep the PE warm through the reciprocal/normalize tail so the
            # final output-projection matmuls run at full clock.
            ps_wt = scp.tile([128, 512], F32, tag="sc", name="tailwarm")
            for _ in range(18):
                nc.tensor.matmul(ps_wt[:, 0:512], warm[:, 0:128], warm[:, :],
                                 start=True, stop=True)
            while sched:
                it_no += 1
                before = len(sched)
                pops()
                if len(sched) == before:
                    sched[0][0] = it_no  # force the oldest item ripe
                    sched[0][4] = it_no

    nc.compile()
    return nc


_PROGRAM = None


def _get_program() -> bass.Bass:
    global _PROGRAM
    if _PROGRAM is None:
        _PROGRAM = _build_program()
    return _PROGRAM


def _bf16(a: np.ndarray) -> np.ndarray:
    return np.ascontiguousarray(a).astype(ml_dtypes.bfloat16)


def _pack_w(wt: np.ndarray, cols: int) -> np.ndarray:
    """[E_contract, cols] -> SBUF tile layout [128, EC_chunks * cols]."""
    k = wt.shape[0]
    return np.ascontiguousarray(
        wt.reshape(k // 128, 128, cols).transpose(1, 0, 2).reshape(128, -1))


def _prepare_in_maps(x, Wq, Wk, Wv, Wo):
    x = np.asarray(x, dtype=np.float32)
    Wq = np.asarray(Wq, dtype=np.float32)
    Wk = np.asarray(Wk, dtype=np.float32)
    Wv = np.asarray(Wv, dtype=np.float32)
    Wo = np.asarray(Wo, dtype=np.float32)
    in_maps = []
    for c in range(NCORES):
        b, hg = c // HGROUPS, c % HGROUPS
        rows = slice(hg * FH, (hg + 1) * FH)
        xt = x[b].T  # [E, S]
        # [SC, 128, EC*512]: per s-chunk, the exact SBUF tile layout.
        xt_tiled = (xt.reshape(EC, 128, SC, 512).transpose(2, 1, 0, 3)
                    .reshape(SC, 128, EC * 512))
        in_maps.append({
            "xt": _bf16(xt_tiled),
            "wqt": _bf16(_pack_w(Wq[rows, :].T, FH)),
            "wkt": _bf16(_pack_w(Wk[rows, :].T, FH)),
            "wvt": _bf16(_pack_w(Wv[rows, :].T, FH)),
            "wot": _bf16(_pack_w(Wo[:, rows].T, E)),
            "ones": np.ones((128, DK), ml_dtypes.bfloat16),
        })
    return in_maps


def run(inputs: dict, **spmd_kwargs):
    """Run on all 8 cores; returns (full output, BassKernelResults)."""
    nc = _get_program()
    in_maps = _prepare_in_maps(**inputs)
    res = run_bass_kernel_spmd(nc, in_maps, core_ids=list(range(NCORES)),
                               **spmd_kwargs)
    partials = [r["y"] for r in res.results]
    out = np.empty((B, S, E), dtype=np.float32)
    for b in range(B):
        acc = partials[b * HGROUPS].astype(np.float32, copy=True)
        for hg in range(1, HGROUPS):
            acc += partials[b * HGROUPS + hg]
        out[b] = acc
    return out, res


def kernel(**inputs) -> np.ndarray:
    out, _ = run(inputs)
    return out
